# revision 1
# baseline (speedup 1.0000x reference)
"""Depthwise 8192-tap temporal conv (NoRollCaTentLayer) on 8 TRN2 cores, v2.

Per-channel correlation via half-spectrum matmul-FFT (L=8192 = 64-DFT x
twiddle x 128-DFT, keeping k2 in [0,32] of 64 by Hermitian symmetry).
All DFT matrices are channel-independent PE stationaries; per-channel
F2 stage uses the x/w data as the stationary so no on-chip transposes
are ever needed.  bf16 operands everywhere (PSUM accum fp32).

Inverse: G = IDFT_128(P) with both-plane rhs [cos|sin] (N=256), inverse
twiddle on DVE, then I2 as block-diagonal stationaries (4 channels x 32
k2 -> 4 channels x 32 v in one [128,128,128] matmul), k2=32 column
handled via a separate [16,...] partition-parallel path.
"""

import os
import sys

sys.path.insert(0, "/opt/trn_rl_repo")

import numpy as np
import ml_dtypes

import concourse.bacc as bacc
import concourse.mybir as mybir
import concourse.tile as tile
from concourse.bass_utils import run_bass_kernel_spmd

T, C, FW, L = 4096, 1024, 8192, 8192
NUM_INH = 256
EPS = 1e-8
NCORES = 8
CPC = C // NCORES          # 128 channels per core
CH = 16                    # channels per round
R = CPC // CH              # 8 rounds
K2 = 33                    # kept half-spectrum columns (k2 = 0..32)
K2M = 32                   # main columns (k2 < 32)

F32 = mybir.dt.float32
BF16 = mybir.dt.bfloat16
BF = ml_dtypes.bfloat16


def _consts():
    a_ = np.arange(128)
    b32 = np.arange(32)
    b64 = np.arange(64)
    k2_ = np.arange(K2)
    k1_ = np.arange(128)
    u_ = np.arange(128)
    v_ = np.arange(32)

    def pk(mat_re, mat_im):  # [rows, 2, cols] (p-major planes)
        return np.stack([mat_re, mat_im], axis=1).astype(BF)

    th2x = 2 * np.pi * np.outer(b32, k2_) / 64
    f2xs = pk(np.cos(th2x), -np.sin(th2x)).reshape(32, 2 * K2)      # [32, 66]
    th2w = 2 * np.pi * np.outer(b64, k2_) / 64
    f2ws = pk(np.cos(th2w), -np.sin(th2w)).reshape(64, 2 * K2)      # [64, 66]

    thtw = 2 * np.pi * np.outer(a_, k2_) / L
    twb = np.stack([np.cos(thtw), -np.sin(thtw)], axis=1)           # [128,2,33]
    twbc = np.broadcast_to(twb[:, :, :, None], (128, 2, K2, 32)).astype(BF)

    th1 = 2 * np.pi * np.outer(a_, k1_) / 128
    f1c = np.stack([np.cos(th1), -np.sin(th1), np.sin(th1)],
                   axis=1).astype(BF)                               # [128,3,128]
    # planes: [cos, sin(=-(-sin)), -sin] wait: we need [cos, sin_n, -sin_n]
    # X = sum yp * (cos - i sin) => Xre = cos@re + sin@im ; Xim = cos@im - sin@re
    # plane0 = cos, plane1 = sin, plane2 = -sin
    f1c = np.stack([np.cos(th1), np.sin(th1), -np.sin(th1)],
                   axis=1).astype(BF)

    thi1 = 2 * np.pi * np.outer(k1_, u_) / 128
    i1cs = np.stack([np.cos(thi1), np.sin(thi1)], axis=1).astype(BF)  # [128,2,128]
    i1cs2 = np.stack([-np.sin(thi1), np.cos(thi1)], axis=1).astype(BF)

    # itwb: partitions (c4, k2<32), free u : e^{+2pi i u k2 / L}
    k2p = np.tile(np.arange(K2M), 4)                                # [128]
    thit = 2 * np.pi * np.outer(k2p, u_) / L
    itwb = np.stack([np.cos(thit), np.sin(thit)], axis=1).astype(BF)  # [128,2,128]

    # iw32: partitions c16, free u: e^{+2pi i u 32 / L} / L  (scale folded)
    th32 = 2 * np.pi * u_ * 32 / L
    iw32 = np.stack([np.broadcast_to(np.cos(th32) / L, (16, 128)),
                     np.broadcast_to(np.sin(th32) / L, (16, 128))],
                    axis=1).astype(BF)                              # [16,2,128]

    # I2 block-diagonal stationaries [128=(c4,k2 32), 128=(c4,v 32)]
    wt = np.ones(K2M); wt[1:] = 2.0
    cmat = (wt[:, None] * np.cos(2 * np.pi * np.outer(np.arange(K2M), v_) / 64)
            / L)                                                    # [k2, v]
    smat = (wt[:, None] * np.sin(2 * np.pi * np.outer(np.arange(K2M), v_) / 64)
            / L)
    i2c = np.zeros((128, 128), np.float32)
    i2sn = np.zeros((128, 128), np.float32)
    for c in range(4):
        i2c[c * 32:(c + 1) * 32, c * 32:(c + 1) * 32] = cmat
        i2sn[c * 32:(c + 1) * 32, c * 32:(c + 1) * 32] = -smat
    i2c = i2c.astype(BF)
    i2sn = i2sn.astype(BF)

    # s32c [16, 4, 128]: for chunk q: S[c16, (c4,v)] = delta_{c,4q+c4} * (-1)^v
    s32c = np.zeros((16, 4, 128), np.float32)
    pmv = ((-1.0) ** v_)
    for q in range(4):
        for c4 in range(4):
            s32c[4 * q + c4, q, c4 * 32:(c4 + 1) * 32] = pmv
    s32c = s32c.astype(BF)

    selp = np.zeros((16, 128), np.float32)
    qmask = np.zeros((16, 4), np.float32)
    for j in range(16):
        selp[j, (j % 4) * 32:(j % 4 + 1) * 32] = 1.0
        qmask[j, j // 4] = 1.0
    ones64 = np.ones((64, 1), np.float32)
    return {
        "f2xs": f2xs, "f2ws": f2ws, "twbc": twbc, "f1c": f1c,
        "i1cs": i1cs, "i1cs2": i1cs2, "itwb": itwb, "iw32": iw32, "i2c": i2c,
        "i2sn": i2sn, "s32c": s32c, "selp": selp, "qmask": qmask,
        "ones64b": ones64.astype(BF),
    }


def _build():
    nc = bacc.Bacc("TRN2", target_bir_lowering=False, debug=False,
                   num_devices=NCORES)
    xp_d = nc.dram_tensor("xprep", [R, 32, CH, 128], BF16, kind="ExternalInput")
    wp_d = nc.dram_tensor("wprep", [R, 64, CH, 128], BF16, kind="ExternalInput")
    f2xs_d = nc.dram_tensor("f2xs", [32, 2 * K2], BF16, kind="ExternalInput")
    f2ws_d = nc.dram_tensor("f2ws", [64, 2 * K2], BF16, kind="ExternalInput")
    twbc_d = nc.dram_tensor("twbc", [128, 2, K2, 32], BF16, kind="ExternalInput")
    f1c_d = nc.dram_tensor("f1c", [128, 3, 128], BF16, kind="ExternalInput")
    i1cs_d = nc.dram_tensor("i1cs", [128, 2, 128], BF16, kind="ExternalInput")
    i1cs2_d = nc.dram_tensor("i1cs2", [128, 2, 128], BF16, kind="ExternalInput")
    itwb_d = nc.dram_tensor("itwb", [128, 2, 128], BF16, kind="ExternalInput")
    iw32_d = nc.dram_tensor("iw32", [16, 2, 128], BF16, kind="ExternalInput")
    i2c_d = nc.dram_tensor("i2c", [128, 128], BF16, kind="ExternalInput")
    i2sn_d = nc.dram_tensor("i2sn", [128, 128], BF16, kind="ExternalInput")
    s32c_d = nc.dram_tensor("s32c", [16, 4, 128], BF16, kind="ExternalInput")
    selp_d = nc.dram_tensor("selp", [16, 128], F32, kind="ExternalInput")
    qmask_d = nc.dram_tensor("qmask", [16, 4], F32, kind="ExternalInput")
    ones64b_d = nc.dram_tensor("ones64b", [64, 1], BF16, kind="ExternalInput")
    bei_d = nc.dram_tensor("beicol", [2, 128, 32], F32, kind="ExternalInput")
    out_d = nc.dram_tensor("outT", [CPC, T], F32, kind="ExternalOutput")
    dbg = os.environ.get("K2_DEBUG", "") == "1"
    if dbg:
        dypc_d = nc.dram_tensor("dypc", [128, 2, K2, 32], F32, kind="ExternalOutput")
        dpre_d = nc.dram_tensor("dpre", [128, 16, K2M], F32, kind="ExternalOutput")
        dpim_d = nc.dram_tensor("dpim", [128, 16, K2M], F32, kind="ExternalOutput")
        dgtre_d = nc.dram_tensor("dgtre", [128, 4, 128], F32, kind="ExternalOutput")
        dgtim_d = nc.dram_tensor("dgtim", [128, 4, 128], F32, kind="ExternalOutput")
        dgt32_d = nc.dram_tensor("dgt32", [16, 128], F32, kind="ExternalOutput")
        dot_d = nc.dram_tensor("dot", [128, 4, 128], F32, kind="ExternalOutput")
        drnq_d = nc.dram_tensor("drnq", [128, 4], F32, kind="ExternalOutput")

    RELU = mybir.ActivationFunctionType.Relu

    with tile.TileContext(nc) as tc:
        with (
            tc.tile_pool(name="sb", bufs=1) as sb,
            tc.tile_pool(name="ps", bufs=1, space="PSUM") as pp,
        ):
            # ---- constants to SBUF ----
            f2xs = sb.tile([32, 2 * K2], BF16, tag="c_f2x")
            nc.sync.dma_start(out=f2xs[:], in_=f2xs_d.ap())
            f2ws = sb.tile([64, 2 * K2], BF16, tag="c_f2w")
            nc.sync.dma_start(out=f2ws[:], in_=f2ws_d.ap())
            twbc = sb.tile([128, 2, K2, 32], BF16, tag="c_twbc")
            nc.sync.dma_start(out=twbc[:], in_=twbc_d.ap())
            f1c = sb.tile([128, 3, 128], BF16, tag="c_f1c")
            nc.sync.dma_start(out=f1c[:], in_=f1c_d.ap())
            i1cs = sb.tile([128, 2, 128], BF16, tag="c_i1cs")
            nc.sync.dma_start(out=i1cs[:], in_=i1cs_d.ap())
            i1cs2 = sb.tile([128, 2, 128], BF16, tag="c_i1cs2")
            nc.sync.dma_start(out=i1cs2[:], in_=i1cs2_d.ap())
            itwb = sb.tile([128, 2, 128], BF16, tag="c_itwb")
            nc.sync.dma_start(out=itwb[:], in_=itwb_d.ap())
            iw32 = sb.tile([16, 2, 128], BF16, tag="c_iw32")
            nc.sync.dma_start(out=iw32[:], in_=iw32_d.ap())
            i2c = sb.tile([128, 128], BF16, tag="c_i2c")
            nc.sync.dma_start(out=i2c[:], in_=i2c_d.ap())
            i2sn = sb.tile([128, 128], BF16, tag="c_i2sn")
            nc.sync.dma_start(out=i2sn[:], in_=i2sn_d.ap())
            s32c = sb.tile([16, 4, 128], BF16, tag="c_s32c")
            nc.sync.dma_start(out=s32c[:], in_=s32c_d.ap())
            selp = sb.tile([16, 128], F32, tag="c_selp")
            nc.sync.dma_start(out=selp[:], in_=selp_d.ap())
            qmask = sb.tile([16, 4], F32, tag="c_qmask")
            nc.sync.dma_start(out=qmask[:], in_=qmask_d.ap())
            ones64b = sb.tile([64, 1], BF16, tag="c_ones")
            nc.sync.dma_start(out=ones64b[:], in_=ones64b_d.ap())
            beis = sb.tile([128, 2, 32], F32, tag="c_beis")
            nc.sync.dma_start(out=beis[:], in_=bei_d.ap().transpose((1, 0, 2)))

            def front(r):
                """loads + relu/norm + F2 + Y-copies + twiddle -> state dict"""
                xs = sb.tile([32, CH, 128], BF16, tag="xs", bufs=3)
                nc.sync.dma_start(out=xs[:], in_=xp_d.ap()[r])
                ws = sb.tile([64, CH, 128], BF16, tag="ws", bufs=3)
                nc.sync.dma_start(out=ws[:], in_=wp_d.ap()[r])

                wsr = sb.tile([64, CH, 128], BF16, tag="wsr", bufs=3)
                nc.vector.tensor_scalar_max(wsr[:], ws[:], 0.0)
                wsq = sb.tile([64, CH, 128], BF16, tag="wsq", bufs=2)
                nc.vector.tensor_mul(wsq[:], ws[:], ws[:])
                sqred = sb.tile([64, CH], BF16, tag="sqred", bufs=2)
                with nc.allow_low_precision(reason="bf16 norm partials"):
                    nc.vector.reduce_sum(sqred[:].unsqueeze(2), wsq[:],
                                         mybir.AxisListType.X)
                nrm2 = pp.tile([CH, 1], F32, tag="y", bufs=2)
                nc.tensor.matmul(nrm2[:], sqred[:], ones64b[:],
                                 start=True, stop=True)
                rn16 = sb.tile([CH, 1], F32, tag="rn16", bufs=3)
                nc.scalar.sqrt(rn16[:], nrm2[:])
                nc.vector.tensor_scalar_max(rn16[:], rn16[:], EPS)
                nc.vector.reciprocal(rn16[:], rn16[:])
                rnmat = sb.tile([16, 4], F32, tag="rnmat", bufs=2)
                nc.vector.tensor_mul(rnmat[:], rn16[:].broadcast_to((16, 4)),
                                     qmask[:])
                rnq = sb.tile([128, 4], F32, tag="rnq", bufs=3)
                rnps = pp.tile([128, 4], F32, tag="y", bufs=2)
                nc.tensor.matmul(rnps[:], selp[:], rnmat[:],
                                 start=True, stop=True)
                nc.scalar.copy(rnq[:], rnps[:])

                yxc = sb.tile([128, 2, K2, 32], BF16, tag="yxc", bufs=3)
                for path in range(2):       # 0 = x, 1 = w
                    src = xs if path == 0 else wsr
                    f2s = f2xs if path == 0 else f2ws
                    c0 = 0
                    for nch in (6, 6, 4):
                        yq = pp.tile([128, 6, 2, K2], F32, tag="y", bufs=2)
                        for c4 in range(nch):
                            nc.tensor.matmul(yq[:, c4], src[:, c0 + c4, :],
                                             f2s[:], start=True, stop=True)
                        cs = 16 * path + c0
                        nc.scalar.copy(yxc[:, :, :, cs:cs + nch],
                                       yq[:, 0:nch].transpose((0, 2, 3, 1)))
                        c0 += nch

                ypc = sb.tile([128, 2, K2, 32], BF16, tag="ypc", bufs=3)
                tt1 = sb.tile([128, K2, 32], BF16, tag="tt", bufs=4)
                tt2 = sb.tile([128, K2, 32], BF16, tag="tt", bufs=4)
                nc.vector.tensor_mul(tt1[:], yxc[:, 0], twbc[:, 0])
                nc.vector.tensor_mul(tt2[:], yxc[:, 1], twbc[:, 1])
                nc.vector.tensor_sub(ypc[:, 0], tt1[:], tt2[:])
                tt3 = sb.tile([128, K2, 32], BF16, tag="tt", bufs=4)
                tt4 = sb.tile([128, K2, 32], BF16, tag="tt", bufs=4)
                nc.vector.tensor_mul(tt3[:], yxc[:, 0], twbc[:, 1])
                nc.vector.tensor_mul(tt4[:], yxc[:, 1], twbc[:, 0])
                nc.vector.tensor_add(ypc[:, 1], tt3[:], tt4[:])
                return {"ypc": ypc, "rnq": rnq}

            def back1(r, st):
                """F1 + psum copies + pointwise -> P tiles"""
                ypc = st["ypc"]
                xw = []
                for path in range(2):
                    cs = slice(16 * path, 16 * path + 16)
                    xmr = pp.tile([128, K2M, 16], F32, tag="xm", bufs=2)
                    nc.tensor.matmul(xmr[:], f1c[:, 0], ypc[:, 0, 0:K2M, cs],
                                     start=True, stop=False)
                    nc.tensor.matmul(xmr[:], f1c[:, 1], ypc[:, 1, 0:K2M, cs],
                                     start=False, stop=True)
                    xmi = pp.tile([128, K2M, 16], F32, tag="xm", bufs=2)
                    nc.tensor.matmul(xmi[:], f1c[:, 0], ypc[:, 1, 0:K2M, cs],
                                     start=True, stop=False)
                    nc.tensor.matmul(xmi[:], f1c[:, 2], ypc[:, 0, 0:K2M, cs],
                                     start=False, stop=True)
                    x32 = pp.tile([128, 2, 16], F32, tag="y", bufs=2)
                    nc.tensor.matmul(x32[:, 0], f1c[:, 0], ypc[:, 0, K2M, cs],
                                     start=True, stop=False)
                    nc.tensor.matmul(x32[:, 0], f1c[:, 1], ypc[:, 1, K2M, cs],
                                     start=False, stop=True)
                    nc.tensor.matmul(x32[:, 1], f1c[:, 0], ypc[:, 1, K2M, cs],
                                     start=True, stop=False)
                    nc.tensor.matmul(x32[:, 1], f1c[:, 2], ypc[:, 0, K2M, cs],
                                     start=False, stop=True)
                    xsb = sb.tile([128, 2, 16, K2], BF16, tag="xsb", bufs=4)
                    nc.scalar.copy(xsb[:, 0, :, 0:K2M],
                                   xmr[:].transpose((0, 2, 1)))
                    nc.scalar.copy(xsb[:, 1, :, 0:K2M],
                                   xmi[:].transpose((0, 2, 1)))
                    nc.scalar.copy(xsb[:, :, :, K2M], x32[:])
                    xw.append(xsb)
                Xs, Ws = xw

                Pre = sb.tile([128, 16, K2M], BF16, tag="Pre", bufs=3)
                Pim = sb.tile([128, 16, K2M], BF16, tag="Pim", bufs=3)
                xm_ = [Xs[:, p, :, 0:K2M] for p in range(2)]
                wm_ = [Ws[:, p, :, 0:K2M] for p in range(2)]
                pp1 = sb.tile([128, 16, K2M], BF16, tag="pp", bufs=4)
                pp2 = sb.tile([128, 16, K2M], BF16, tag="pp", bufs=4)
                nc.vector.tensor_mul(pp1[:], xm_[0], wm_[0])
                nc.vector.tensor_mul(pp2[:], xm_[1], wm_[1])
                nc.vector.tensor_add(Pre[:], pp1[:], pp2[:])
                pp3 = sb.tile([128, 16, K2M], BF16, tag="pp", bufs=4)
                pp4 = sb.tile([128, 16, K2M], BF16, tag="pp", bufs=4)
                nc.vector.tensor_mul(pp3[:], xm_[1], wm_[0])
                nc.vector.tensor_mul(pp4[:], xm_[0], wm_[1])
                nc.vector.tensor_sub(Pim[:], pp3[:], pp4[:])
                p32 = sb.tile([128, 2, 16], BF16, tag="p32", bufs=3)
                q1 = sb.tile([128, 2, 16], BF16, tag="pq", bufs=4)
                q2 = sb.tile([128, 2, 16], BF16, tag="pq", bufs=4)
                nc.vector.tensor_mul(q1[:, 0], Xs[:, 0, :, K2M], Ws[:, 0, :, K2M])
                nc.vector.tensor_mul(q2[:, 0], Xs[:, 1, :, K2M], Ws[:, 1, :, K2M])
                nc.vector.tensor_add(p32[:, 0], q1[:, 0], q2[:, 0])
                nc.vector.tensor_mul(q1[:, 1], Xs[:, 1, :, K2M], Ws[:, 0, :, K2M])
                nc.vector.tensor_mul(q2[:, 1], Xs[:, 0, :, K2M], Ws[:, 1, :, K2M])
                nc.vector.tensor_sub(p32[:, 1], q1[:, 1], q2[:, 1])
                return {"Pre": Pre, "Pim": Pim, "p32": p32}

            def back2(r, mid, st):
                """G/itw + G32 + I2 + activation + store"""
                Pre, Pim, p32 = mid["Pre"], mid["Pim"], mid["p32"]
                rnq = st["rnq"]
                gtre = sb.tile([128, 4, 128], BF16, tag="gtre", bufs=2)
                gtim = sb.tile([128, 4, 128], BF16, tag="gtim", bufs=2)
                for pair in range(2):
                    gt = pp.tile([128, 2, 2, 128], F32, tag="ab", bufs=2)
                    for h in range(2):
                        q = 2 * pair + h
                        nc.tensor.matmul(gt[:, h], Pre[:, 4 * q:4 * q + 4],
                                         i1cs[:], start=True, stop=False)
                        nc.tensor.matmul(gt[:, h], Pim[:, 4 * q:4 * q + 4],
                                         i1cs2[:], start=False, stop=True)
                    gts = sb.tile([128, 2, 2, 128], BF16, tag="gts", bufs=4)
                    nc.scalar.copy(gts[:], gt[:])
                    qs = slice(2 * pair, 2 * pair + 2)
                    iwr = itwb[:, 0].unsqueeze(1).broadcast_to((128, 2, 128))
                    iwi = itwb[:, 1].unsqueeze(1).broadcast_to((128, 2, 128))
                    gq1 = sb.tile([128, 2, 128], BF16, tag="gq", bufs=4)
                    gq2 = sb.tile([128, 2, 128], BF16, tag="gq", bufs=4)
                    nc.vector.tensor_mul(gq1[:], gts[:, :, 0], iwr)
                    nc.vector.tensor_mul(gq2[:], gts[:, :, 1], iwi)
                    nc.vector.tensor_sub(gtre[:, qs], gq1[:], gq2[:])
                    gq3 = sb.tile([128, 2, 128], BF16, tag="gq", bufs=4)
                    gq4 = sb.tile([128, 2, 128], BF16, tag="gq", bufs=4)
                    nc.vector.tensor_mul(gq3[:], gts[:, :, 0], iwi)
                    nc.vector.tensor_mul(gq4[:], gts[:, :, 1], iwr)
                    nc.vector.tensor_add(gtim[:, qs], gq3[:], gq4[:])

                ab32 = pp.tile([16, 2, 128], F32, tag="ab32", bufs=1)
                nc.tensor.matmul(ab32[:], p32[:, 0], i1cs[:],
                                 start=True, stop=False)
                nc.tensor.matmul(ab32[:], p32[:, 1], i1cs2[:],
                                 start=False, stop=True)
                g32a = sb.tile([16, 128], BF16, tag="g32t", bufs=4)
                g32b = sb.tile([16, 128], BF16, tag="g32t", bufs=4)
                nc.vector.tensor_mul(g32a[:], ab32[:, 0], iw32[:, 0])
                nc.vector.tensor_mul(g32b[:], ab32[:, 1], iw32[:, 1])
                gt32 = sb.tile([16, 128], BF16, tag="gt32", bufs=2)
                nc.vector.tensor_sub(gt32[:], g32a[:], g32b[:])

                ot = pp.tile([128, 4, 128], F32, tag="o", bufs=1)
                for q in range(4):
                    nc.tensor.matmul(ot[:, q], i2c[:], gtre[:, q],
                                     start=True, stop=False)
                    nc.tensor.matmul(ot[:, q], i2sn[:], gtim[:, q],
                                     start=False, stop=False)
                    nc.tensor.matmul(ot[:, q], s32c[:, q], gt32[:],
                                     start=False, stop=True)
                outm = sb.tile([128, 4, 128], F32, tag="outm", bufs=2)
                for q in range(4):
                    j = 4 * r + q
                    nc.scalar.activation(outm[:, q], ot[:, q], RELU,
                                         scale=rnq[:, q:q + 1],
                                         bias=beis[:, 0, j:j + 1])
                    nc.scalar.mul(outm[:, q], outm[:, q],
                                  beis[:, 1, j:j + 1])
                nc.sync.dma_start(
                    out=out_d.ap()[CH * r:CH * r + 16].rearrange(
                        "(q c) (v u) -> (c v) q u", q=4, v=32),
                    in_=outm[:])

            # 3-stage software pipeline:
            # F(0) F(1) B1(0) | F(2) B2(0) B1(1) | F(3) B2(1) B1(2) | ...
            sts = {0: front(0), 1: front(1)}
            mids = {0: back1(0, sts[0])}
            for r in range(R):
                if r + 2 < R:
                    sts[r + 2] = front(r + 2)
                back2(r, mids[r], sts[r])
                if r + 1 < R:
                    mids[r + 1] = back1(r + 1, sts[r + 1])

    nc.compile()
    return nc


_CACHE = {}


def _prep(x, w, b):
    """Host-side sharding + layout prep (per core)."""
    ei = np.concatenate([np.ones(C - NUM_INH, np.float32),
                         -np.ones(NUM_INH, np.float32)])
    consts = _CACHE["consts"]
    in_maps = []
    for i in range(NCORES):
        sl = slice(CPC * i, CPC * (i + 1))
        xs = x[:, sl]                        # [T, CPC]
        wsl = w[:, sl]                       # [FW, CPC]
        # circular arrangement wcr[m]: m<4096: w[m+4095]; m==4096: w[8191];
        # m>4096: w[m-4097]   (slot 4096 unused by conv; holds w[8191] so
        # that sum(wcr^2) == ||w||^2 exactly)
        wcr = np.empty((L, CPC), np.float32)
        wcr[:T] = wsl[T - 1:2 * T - 1]
        wcr[T] = wsl[2 * T - 1]
        wcr[T + 1:] = wsl[0:T - 1]
        # xprep [R, 32, CH, 128]: t = a + 128 b ; channel ch = CH*r + c
        xprep = np.ascontiguousarray(
            xs.reshape(32, 128, R, CH).transpose(2, 0, 3, 1).astype(BF))
        wprep = np.ascontiguousarray(
            wcr.reshape(64, 128, R, CH).transpose(2, 0, 3, 1).astype(BF))
        # beicol [2, 128, 32]: [which, (c4,v), j=4r+q] -> channel CH*r+4q+c4
        bcol = np.zeros((128, 32), np.float32)
        ecol = np.zeros((128, 32), np.float32)
        bsl = b[sl]
        esl = ei[sl]
        for j in range(32):
            rr, q = j // 4, j % 4
            for c4 in range(4):
                ch = CH * rr + 4 * q + c4
                bcol[c4 * 32:(c4 + 1) * 32, j] = bsl[ch]
                ecol[c4 * 32:(c4 + 1) * 32, j] = esl[ch]
        m = {"xprep": xprep, "wprep": wprep,
             "beicol": np.ascontiguousarray(np.stack([bcol, ecol]))}
        m.update(consts)
        in_maps.append(m)
    return in_maps


def kernel(x, w, b):
    if "nc" not in _CACHE:
        _CACHE["consts"] = _consts()
        _CACHE["nc"] = _build()
    nc = _CACHE["nc"]

    x = np.ascontiguousarray(np.asarray(x, dtype=np.float32))
    w = np.ascontiguousarray(np.asarray(w, dtype=np.float32))
    b = np.ascontiguousarray(np.asarray(b, dtype=np.float32))
    in_maps = _prep(x, w, b)

    trace = os.environ.get("BASS_CONV_TRACE", "") == "1"
    last_err = None
    for _attempt in range(3):
        try:
            res = run_bass_kernel_spmd(nc, in_maps, core_ids=list(range(NCORES)),
                                       trace=trace)
            break
        except Exception as e:   # transient NRT device errors happen under axon
            last_err = e
    else:
        raise last_err
    if trace and res.exec_time_ns is not None:
        print(f"HW exec time: {res.exec_time_ns} ns")
        kernel.last_exec_ns = res.exec_time_ns
    out = np.concatenate(
        [np.ascontiguousarray(res.results[i]["outT"].T) for i in range(NCORES)],
        axis=1)
    return out



# revision 2
# speedup vs baseline: 1.0162x; 1.0162x over previous
"""Depthwise 8192-tap temporal conv (NoRollCaTentLayer) on 8 TRN2 cores, v3.

Per-channel correlation via half-spectrum matmul-FFT (L=8192 = 64-DFT x
twiddle x 128-DFT, keeping k2 in [0,32] of 64 by Hermitian symmetry).

v3 over v2:
 - EI mask (+-1 per 128-ch core slice) applied host-side at gather.
 - F2 stage channel-batched: 4-ch (x) / 2-ch (w) data blocks as one
   [128,128] PE stationary against block-diagonal DFT moving operands
   (12 matmuls/round instead of 32).
 - w-norm path on a [128, CH, 64] layout (full-partition DVE ops)
   instead of [64, CH, 128].
 - 1/||w|| folded into the I1->SBUF copies (Act scale) and the k2=32
   path (scalar_tensor_tensor), so the final activation is bias+relu
   only.
 - I2 matmuls merged across the 4 q-blocks (6 matmuls instead of 12);
   inverse twiddle done in 6 all-q DVE ops instead of 12.
"""

import os
import sys

sys.path.insert(0, "/opt/trn_rl_repo")

import numpy as np
import ml_dtypes

import concourse.bacc as bacc
import concourse.mybir as mybir
import concourse.tile as tile
from concourse.bass_utils import run_bass_kernel_spmd

T, C, FW, L = 4096, 1024, 8192, 8192
NUM_INH = 256
EPS = 1e-8
NCORES = 8
CPC = C // NCORES          # 128 channels per core
CH = 16                    # channels per round
R = CPC // CH              # 8 rounds
K2 = 33                    # kept half-spectrum columns (k2 = 0..32)
K2M = 32                   # main columns (k2 < 32)

F32 = mybir.dt.float32
BF16 = mybir.dt.bfloat16
BF = ml_dtypes.bfloat16


def _consts():
    a_ = np.arange(128)
    b32 = np.arange(32)
    b64 = np.arange(64)
    k2_ = np.arange(K2)
    k1_ = np.arange(128)
    u_ = np.arange(128)
    v_ = np.arange(32)

    def pk(mat_re, mat_im):  # [rows, 2, cols] (p-major planes)
        return np.stack([mat_re, mat_im], axis=1)

    th2x = 2 * np.pi * np.outer(b32, k2_) / 64
    f2xs = pk(np.cos(th2x), -np.sin(th2x)).reshape(32, 2 * K2)      # [32, 66]
    th2w = 2 * np.pi * np.outer(b64, k2_) / 64
    f2ws = pk(np.cos(th2w), -np.sin(th2w)).reshape(64, 2 * K2)      # [64, 66]

    # block-diagonal moving operands for the channel-batched F2 stage
    f2xb = np.zeros((128, 4, 2 * K2), np.float32)                   # [(c,m),c',j]
    for c in range(4):
        f2xb[32 * c:32 * c + 32, c] = f2xs
    f2wb = np.zeros((128, 2, 2 * K2), np.float32)
    for c in range(2):
        f2wb[64 * c:64 * c + 64, c] = f2ws
    f2xb = f2xb.astype(BF)
    f2wb = f2wb.astype(BF)

    thtw = 2 * np.pi * np.outer(a_, k2_) / L
    twb = np.stack([np.cos(thtw), -np.sin(thtw)], axis=1)           # [128,2,33]
    twbc = np.broadcast_to(twb[:, :, :, None], (128, 2, K2, 32)).astype(BF)

    th1 = 2 * np.pi * np.outer(a_, k1_) / 128
    # planes: [cos, sin, -sin]
    f1c = np.stack([np.cos(th1), np.sin(th1), -np.sin(th1)],
                   axis=1).astype(BF)                               # [128,3,128]

    thi1 = 2 * np.pi * np.outer(k1_, u_) / 128
    i1cs = np.stack([np.cos(thi1), np.sin(thi1)], axis=1).astype(BF)  # [128,2,128]
    i1cs2 = np.stack([-np.sin(thi1), np.cos(thi1)], axis=1).astype(BF)

    # itwb: partitions (c4, k2<32), free u : e^{+2pi i u k2 / L}
    k2p = np.tile(np.arange(K2M), 4)                                # [128]
    thit = 2 * np.pi * np.outer(k2p, u_) / L
    itwb = np.stack([np.cos(thit), np.sin(thit)], axis=1).astype(BF)  # [128,2,128]

    # iw32: partitions c16, free u: e^{+2pi i u 32 / L} / L  (scale folded)
    th32 = 2 * np.pi * u_ * 32 / L
    iw32 = np.stack([np.broadcast_to(np.cos(th32) / L, (16, 128)),
                     np.broadcast_to(np.sin(th32) / L, (16, 128))],
                    axis=1).astype(BF)                              # [16,2,128]

    # I2 block-diagonal stationaries [128=(c4,k2 32), 128=(c4,v 32)]
    wt = np.ones(K2M)
    wt[1:] = 2.0
    cmat = (wt[:, None] * np.cos(2 * np.pi * np.outer(np.arange(K2M), v_) / 64)
            / L)                                                    # [k2, v]
    smat = (wt[:, None] * np.sin(2 * np.pi * np.outer(np.arange(K2M), v_) / 64)
            / L)
    i2c = np.zeros((128, 128), np.float32)
    i2sn = np.zeros((128, 128), np.float32)
    for c in range(4):
        i2c[c * 32:(c + 1) * 32, c * 32:(c + 1) * 32] = cmat
        i2sn[c * 32:(c + 1) * 32, c * 32:(c + 1) * 32] = -smat
    i2c = i2c.astype(BF)
    i2sn = i2sn.astype(BF)

    # s32c [16, 4, 128]: for chunk q: S[c16, (c4,v)] = delta_{c,4q+c4} * (-1)^v
    s32c = np.zeros((16, 4, 128), np.float32)
    pmv = ((-1.0) ** v_)
    for q in range(4):
        for c4 in range(4):
            s32c[4 * q + c4, q, c4 * 32:(c4 + 1) * 32] = pmv
    s32c = s32c.astype(BF)

    selp = np.zeros((16, 128), np.float32)
    qmask = np.zeros((16, 4), np.float32)
    for j in range(16):
        selp[j, (j % 4) * 32:(j % 4 + 1) * 32] = 1.0
        qmask[j, j // 4] = 1.0
    ones128 = np.ones((128, 1), np.float32)
    return {
        "f2xb": f2xb, "f2wb": f2wb, "twbc": twbc, "f1c": f1c,
        "i1cs": i1cs, "i1cs2": i1cs2, "itwb": itwb, "iw32": iw32, "i2c": i2c,
        "i2sn": i2sn, "s32c": s32c, "selp": selp, "qmask": qmask,
        "ones128b": ones128.astype(BF),
    }


def _build():
    nc = bacc.Bacc("TRN2", target_bir_lowering=False, debug=False,
                   num_devices=NCORES)
    xp_d = nc.dram_tensor("xprep", [R, 128, 4, 128], BF16, kind="ExternalInput")
    wp_d = nc.dram_tensor("wprep", [R, 128, 8, 128], BF16, kind="ExternalInput")
    wn_d = nc.dram_tensor("wnp", [R, 128, CH, 64], BF16, kind="ExternalInput")
    f2xb_d = nc.dram_tensor("f2xb", [128, 4, 2 * K2], BF16, kind="ExternalInput")
    f2wb_d = nc.dram_tensor("f2wb", [128, 2, 2 * K2], BF16, kind="ExternalInput")
    twbc_d = nc.dram_tensor("twbc", [128, 2, K2, 32], BF16, kind="ExternalInput")
    f1c_d = nc.dram_tensor("f1c", [128, 3, 128], BF16, kind="ExternalInput")
    i1cs_d = nc.dram_tensor("i1cs", [128, 2, 128], BF16, kind="ExternalInput")
    i1cs2_d = nc.dram_tensor("i1cs2", [128, 2, 128], BF16, kind="ExternalInput")
    itwb_d = nc.dram_tensor("itwb", [128, 2, 128], BF16, kind="ExternalInput")
    iw32_d = nc.dram_tensor("iw32", [16, 2, 128], BF16, kind="ExternalInput")
    i2c_d = nc.dram_tensor("i2c", [128, 128], BF16, kind="ExternalInput")
    i2sn_d = nc.dram_tensor("i2sn", [128, 128], BF16, kind="ExternalInput")
    s32c_d = nc.dram_tensor("s32c", [16, 4, 128], BF16, kind="ExternalInput")
    selp_d = nc.dram_tensor("selp", [16, 128], F32, kind="ExternalInput")
    qmask_d = nc.dram_tensor("qmask", [16, 4], F32, kind="ExternalInput")
    ones128b_d = nc.dram_tensor("ones128b", [128, 1], BF16, kind="ExternalInput")
    bcol_d = nc.dram_tensor("bcol", [128, 32], F32, kind="ExternalInput")
    out_d = nc.dram_tensor("outT", [CPC, T], F32, kind="ExternalOutput")

    RELU = mybir.ActivationFunctionType.Relu
    MUL = mybir.AluOpType.mult

    with tile.TileContext(nc) as tc:
        with (
            tc.tile_pool(name="sb", bufs=1) as sb,
            tc.tile_pool(name="ps", bufs=1, space="PSUM") as pp,
        ):
            # ---- constants to SBUF ----
            f2xb = sb.tile([128, 4, 2 * K2], BF16, tag="c_f2x")
            nc.sync.dma_start(out=f2xb[:], in_=f2xb_d.ap())
            f2wb = sb.tile([128, 2, 2 * K2], BF16, tag="c_f2w")
            nc.sync.dma_start(out=f2wb[:], in_=f2wb_d.ap())
            twbc = sb.tile([128, 2, K2, 32], BF16, tag="c_twbc")
            nc.sync.dma_start(out=twbc[:], in_=twbc_d.ap())
            f1c = sb.tile([128, 3, 128], BF16, tag="c_f1c")
            nc.sync.dma_start(out=f1c[:], in_=f1c_d.ap())
            i1cs = sb.tile([128, 2, 128], BF16, tag="c_i1cs")
            nc.sync.dma_start(out=i1cs[:], in_=i1cs_d.ap())
            i1cs2 = sb.tile([128, 2, 128], BF16, tag="c_i1cs2")
            nc.sync.dma_start(out=i1cs2[:], in_=i1cs2_d.ap())
            itwb = sb.tile([128, 2, 128], BF16, tag="c_itwb")
            nc.sync.dma_start(out=itwb[:], in_=itwb_d.ap())
            iw32 = sb.tile([16, 2, 128], BF16, tag="c_iw32")
            nc.sync.dma_start(out=iw32[:], in_=iw32_d.ap())
            i2c = sb.tile([128, 128], BF16, tag="c_i2c")
            nc.sync.dma_start(out=i2c[:], in_=i2c_d.ap())
            i2sn = sb.tile([128, 128], BF16, tag="c_i2sn")
            nc.sync.dma_start(out=i2sn[:], in_=i2sn_d.ap())
            s32c = sb.tile([16, 4, 128], BF16, tag="c_s32c")
            nc.sync.dma_start(out=s32c[:], in_=s32c_d.ap())
            selp = sb.tile([16, 128], F32, tag="c_selp")
            nc.sync.dma_start(out=selp[:], in_=selp_d.ap())
            qmask = sb.tile([16, 4], F32, tag="c_qmask")
            nc.sync.dma_start(out=qmask[:], in_=qmask_d.ap())
            ones128b = sb.tile([128, 1], BF16, tag="c_ones")
            nc.sync.dma_start(out=ones128b[:], in_=ones128b_d.ap())
            beis = sb.tile([128, 32], F32, tag="c_beis")
            nc.sync.dma_start(out=beis[:], in_=bcol_d.ap())

            def front(r):
                """loads + relu/norm + F2 + Y-copies + twiddle -> state dict"""
                xs = sb.tile([128, 4, 128], BF16, tag="xs", bufs=3)
                nc.sync.dma_start(out=xs[:], in_=xp_d.ap()[r])
                ws = sb.tile([128, 8, 128], BF16, tag="ws", bufs=3)
                nc.sync.dma_start(out=ws[:], in_=wp_d.ap()[r])
                wn = sb.tile([128, CH, 64], BF16, tag="wn", bufs=3)
                nc.sync.dma_start(out=wn[:], in_=wn_d.ap()[r])

                wsr = sb.tile([128, 8, 128], BF16, tag="wsr", bufs=3)
                nc.vector.tensor_scalar_max(wsr[:], ws[:], 0.0)
                wq = sb.tile([128, CH, 64], BF16, tag="wq", bufs=2)
                nc.vector.tensor_mul(wq[:], wn[:], wn[:])
                sq128 = sb.tile([128, CH], BF16, tag="sq128", bufs=2)
                with nc.allow_low_precision(reason="bf16 norm partials"):
                    nc.vector.reduce_sum(sq128[:].unsqueeze(2), wq[:],
                                         mybir.AxisListType.X)
                nrm2 = pp.tile([CH, 1], F32, tag="pea", bufs=2)
                nc.tensor.matmul(nrm2[:], sq128[:], ones128b[:],
                                 start=True, stop=True)
                rn16 = sb.tile([CH, 1], F32, tag="rn16", bufs=3)
                nc.scalar.sqrt(rn16[:], nrm2[:])
                nc.vector.tensor_scalar_max(rn16[:], rn16[:], EPS)
                nc.vector.reciprocal(rn16[:], rn16[:])
                rnmat = sb.tile([16, 4], F32, tag="rnmat", bufs=2)
                nc.vector.tensor_mul(rnmat[:], rn16[:].broadcast_to((16, 4)),
                                     qmask[:])
                rnq = sb.tile([128, 4], F32, tag="rnq", bufs=3)
                rnps = pp.tile([128, 4], F32, tag="pea", bufs=2)
                nc.tensor.matmul(rnps[:], selp[:], rnmat[:],
                                 start=True, stop=True)
                nc.scalar.copy(rnq[:], rnps[:])

                yxc = sb.tile([128, 2, K2, 32], BF16, tag="yxc", bufs=3)
                # x path: 4 batched matmuls (4 channels each)
                for g in range(4):
                    yqx = pp.tile([128, 4, 2, K2], F32, tag="pea", bufs=2)
                    nc.tensor.matmul(yqx[:], xs[:, g, :], f2xb[:],
                                     start=True, stop=True)
                    nc.scalar.copy(yxc[:, :, :, 4 * g:4 * g + 4],
                                   yqx[:].transpose((0, 2, 3, 1)))
                # w path: 8 batched matmuls (2 channels each), 3 per bank
                h0 = 0
                for nh in (3, 3, 2):
                    yqw = pp.tile([128, 3, 2, 2, K2], F32, tag="pea", bufs=2)
                    for j in range(nh):
                        nc.tensor.matmul(yqw[:, j], wsr[:, h0 + j, :],
                                         f2wb[:], start=True, stop=True)
                    cs = 16 + 2 * h0
                    dst = yxc[:, :, :, cs:cs + 2 * nh].rearrange(
                        "p a k (j c) -> p a k j c", j=nh)
                    nc.scalar.copy(dst, yqw[:, 0:nh].transpose((0, 3, 4, 1, 2)))
                    h0 += nh

                ypc = sb.tile([128, 2, K2, 32], BF16, tag="ypc", bufs=3)
                tt1 = sb.tile([128, K2, 32], BF16, tag="tt", bufs=4)
                tt2 = sb.tile([128, K2, 32], BF16, tag="tt", bufs=4)
                nc.vector.tensor_mul(tt1[:], yxc[:, 0], twbc[:, 0])
                nc.vector.tensor_mul(tt2[:], yxc[:, 1], twbc[:, 1])
                nc.vector.tensor_sub(ypc[:, 0], tt1[:], tt2[:])
                tt3 = sb.tile([128, K2, 32], BF16, tag="tt", bufs=4)
                tt4 = sb.tile([128, K2, 32], BF16, tag="tt", bufs=4)
                nc.vector.tensor_mul(tt3[:], yxc[:, 0], twbc[:, 1])
                nc.vector.tensor_mul(tt4[:], yxc[:, 1], twbc[:, 0])
                nc.vector.tensor_add(ypc[:, 1], tt3[:], tt4[:])
                return {"ypc": ypc, "rnq": rnq, "rn16": rn16}

            def back1(r, st):
                """F1 + psum copies + pointwise -> P tiles"""
                ypc = st["ypc"]
                xw = []
                for path in range(2):
                    cs = slice(16 * path, 16 * path + 16)
                    xmr = pp.tile([128, K2M, 16], F32, tag="xm", bufs=2)
                    nc.tensor.matmul(xmr[:], f1c[:, 0], ypc[:, 0, 0:K2M, cs],
                                     start=True, stop=False)
                    nc.tensor.matmul(xmr[:], f1c[:, 1], ypc[:, 1, 0:K2M, cs],
                                     start=False, stop=True)
                    xmi = pp.tile([128, K2M, 16], F32, tag="xm", bufs=2)
                    nc.tensor.matmul(xmi[:], f1c[:, 0], ypc[:, 1, 0:K2M, cs],
                                     start=True, stop=False)
                    nc.tensor.matmul(xmi[:], f1c[:, 2], ypc[:, 0, 0:K2M, cs],
                                     start=False, stop=True)
                    x32 = pp.tile([128, 2, 16], F32, tag="pea", bufs=2)
                    nc.tensor.matmul(x32[:, 0], f1c[:, 0], ypc[:, 0, K2M, cs],
                                     start=True, stop=False)
                    nc.tensor.matmul(x32[:, 0], f1c[:, 1], ypc[:, 1, K2M, cs],
                                     start=False, stop=True)
                    nc.tensor.matmul(x32[:, 1], f1c[:, 0], ypc[:, 1, K2M, cs],
                                     start=True, stop=False)
                    nc.tensor.matmul(x32[:, 1], f1c[:, 2], ypc[:, 0, K2M, cs],
                                     start=False, stop=True)
                    xsb = sb.tile([128, 2, 16, K2], BF16, tag="xsb", bufs=4)
                    nc.scalar.copy(xsb[:, 0, :, 0:K2M],
                                   xmr[:].transpose((0, 2, 1)))
                    nc.scalar.copy(xsb[:, 1, :, 0:K2M],
                                   xmi[:].transpose((0, 2, 1)))
                    nc.scalar.copy(xsb[:, :, :, K2M], x32[:])
                    xw.append(xsb)
                Xs, Ws = xw

                Pre = sb.tile([128, 16, K2M], BF16, tag="Pre", bufs=3)
                Pim = sb.tile([128, 16, K2M], BF16, tag="Pim", bufs=3)
                xm_ = [Xs[:, p, :, 0:K2M] for p in range(2)]
                wm_ = [Ws[:, p, :, 0:K2M] for p in range(2)]
                pp1 = sb.tile([128, 16, K2M], BF16, tag="pp", bufs=4)
                pp2 = sb.tile([128, 16, K2M], BF16, tag="pp", bufs=4)
                nc.vector.tensor_mul(pp1[:], xm_[0], wm_[0])
                nc.vector.tensor_mul(pp2[:], xm_[1], wm_[1])
                nc.vector.tensor_add(Pre[:], pp1[:], pp2[:])
                pp3 = sb.tile([128, 16, K2M], BF16, tag="pp", bufs=4)
                pp4 = sb.tile([128, 16, K2M], BF16, tag="pp", bufs=4)
                nc.vector.tensor_mul(pp3[:], xm_[1], wm_[0])
                nc.vector.tensor_mul(pp4[:], xm_[0], wm_[1])
                nc.vector.tensor_sub(Pim[:], pp3[:], pp4[:])
                p32 = sb.tile([128, 2, 16], BF16, tag="p32", bufs=3)
                q1 = sb.tile([128, 2, 16], BF16, tag="pq", bufs=4)
                q2 = sb.tile([128, 2, 16], BF16, tag="pq", bufs=4)
                nc.vector.tensor_mul(q1[:, 0], Xs[:, 0, :, K2M], Ws[:, 0, :, K2M])
                nc.vector.tensor_mul(q2[:, 0], Xs[:, 1, :, K2M], Ws[:, 1, :, K2M])
                nc.vector.tensor_add(p32[:, 0], q1[:, 0], q2[:, 0])
                nc.vector.tensor_mul(q1[:, 1], Xs[:, 1, :, K2M], Ws[:, 0, :, K2M])
                nc.vector.tensor_mul(q2[:, 1], Xs[:, 0, :, K2M], Ws[:, 1, :, K2M])
                nc.vector.tensor_sub(p32[:, 1], q1[:, 1], q2[:, 1])
                return {"Pre": Pre, "Pim": Pim, "p32": p32}

            def back2(r, mid, st):
                """G/itw + G32 + I2 + activation + store"""
                Pre, Pim, p32 = mid["Pre"], mid["Pim"], mid["p32"]
                rnq, rn16 = st["rnq"], st["rn16"]
                gts4 = sb.tile([128, 4, 2, 128], BF16, tag="gts", bufs=2)
                for pair in range(2):
                    gt = pp.tile([128, 2, 2, 128], F32, tag="gt", bufs=2)
                    for h in range(2):
                        q = 2 * pair + h
                        nc.tensor.matmul(gt[:, h], Pre[:, 4 * q:4 * q + 4],
                                         i1cs[:], start=True, stop=False)
                        nc.tensor.matmul(gt[:, h], Pim[:, 4 * q:4 * q + 4],
                                         i1cs2[:], start=False, stop=True)
                    for h in range(2):
                        q = 2 * pair + h
                        nc.scalar.mul(gts4[:, q], gt[:, h], rnq[:, q:q + 1])

                ab32 = pp.tile([16, 2, 128], F32, tag="pea", bufs=2)
                nc.tensor.matmul(ab32[:], p32[:, 0], i1cs[:],
                                 start=True, stop=False)
                nc.tensor.matmul(ab32[:], p32[:, 1], i1cs2[:],
                                 start=False, stop=True)
                g32a = sb.tile([16, 128], BF16, tag="g32t", bufs=4)
                g32b = sb.tile([16, 128], BF16, tag="g32t", bufs=4)
                nc.vector.scalar_tensor_tensor(g32a[:], ab32[:, 0],
                                               rn16[:, 0:1], iw32[:, 0],
                                               op0=MUL, op1=MUL)
                nc.vector.scalar_tensor_tensor(g32b[:], ab32[:, 1],
                                               rn16[:, 0:1], iw32[:, 1],
                                               op0=MUL, op1=MUL)
                gt32 = sb.tile([16, 128], BF16, tag="gt32", bufs=2)
                nc.vector.tensor_sub(gt32[:], g32a[:], g32b[:])

                # inverse twiddle, all 4 q-blocks at once
                iwr = itwb[:, 0].unsqueeze(1).broadcast_to((128, 4, 128))
                iwi = itwb[:, 1].unsqueeze(1).broadcast_to((128, 4, 128))
                gtre = sb.tile([128, 4, 128], BF16, tag="gtre", bufs=2)
                gtim = sb.tile([128, 4, 128], BF16, tag="gtim", bufs=2)
                gq1 = sb.tile([128, 4, 128], BF16, tag="gq", bufs=4)
                gq2 = sb.tile([128, 4, 128], BF16, tag="gq", bufs=4)
                nc.vector.tensor_mul(gq1[:], gts4[:, :, 0], iwr)
                nc.vector.tensor_mul(gq2[:], gts4[:, :, 1], iwi)
                nc.vector.tensor_sub(gtre[:], gq1[:], gq2[:])
                gq3 = sb.tile([128, 4, 128], BF16, tag="gq", bufs=4)
                gq4 = sb.tile([128, 4, 128], BF16, tag="gq", bufs=4)
                nc.vector.tensor_mul(gq3[:], gts4[:, :, 0], iwi)
                nc.vector.tensor_mul(gq4[:], gts4[:, :, 1], iwr)
                nc.vector.tensor_add(gtim[:], gq3[:], gq4[:])

                ot = pp.tile([128, 4, 128], F32, tag="o", bufs=1)
                nc.tensor.matmul(ot[:], i2c[:], gtre[:],
                                 start=True, stop=False)
                nc.tensor.matmul(ot[:], i2sn[:], gtim[:],
                                 start=False, stop=False)
                for q in range(4):
                    nc.tensor.matmul(ot[:, q], s32c[:, q], gt32[:],
                                     start=False, stop=True)
                outm = sb.tile([128, 4, 128], F32, tag="outm", bufs=2)
                for q in range(4):
                    j = 4 * r + q
                    nc.scalar.activation(outm[:, q], ot[:, q], RELU,
                                         bias=beis[:, j:j + 1])
                nc.sync.dma_start(
                    out=out_d.ap()[CH * r:CH * r + 16].rearrange(
                        "(q c) (v u) -> (c v) q u", q=4, v=32),
                    in_=outm[:])

            # 3-stage software pipeline:
            # F(0) F(1) B1(0) | F(2) B2(0) B1(1) | F(3) B2(1) B1(2) | ...
            sts = {0: front(0), 1: front(1)}
            mids = {0: back1(0, sts[0])}
            for r in range(R):
                if r + 2 < R:
                    sts[r + 2] = front(r + 2)
                back2(r, mids[r], sts[r])
                if r + 1 < R:
                    mids[r + 1] = back1(r + 1, sts[r + 1])

    nc.compile()
    return nc


_CACHE = {}


def _prep(x, w, b):
    """Host-side sharding + layout prep (per core)."""
    consts = _CACHE["consts"]
    in_maps = []
    for i in range(NCORES):
        sl = slice(CPC * i, CPC * (i + 1))
        xs = x[:, sl]                        # [T, CPC]
        wsl = w[:, sl]                       # [FW, CPC]
        # circular arrangement wcr[m]: m<4096: w[m+4095]; m==4096: w[8191];
        # m>4096: w[m-4097]   (slot 4096 unused by conv; holds w[8191] so
        # that sum(wcr^2) == ||w||^2 exactly)
        wcr = np.empty((L, CPC), np.float32)
        wcr[:T] = wsl[T - 1:2 * T - 1]
        wcr[T] = wsl[2 * T - 1]
        wcr[T + 1:] = wsl[0:T - 1]
        # xprep [R, 128=(c4,m32), 4=g, 128=b]: ch = 16r + 4g + c4, t = 128m + b
        xprep = np.ascontiguousarray(
            xs.reshape(32, 128, R, 4, 4).transpose(2, 4, 0, 3, 1)
            .reshape(R, 128, 4, 128).astype(BF))
        # wprep [R, 128=(c2,m64), 8=h, 128=b]: ch = 16r + 2h + c2
        wprep = np.ascontiguousarray(
            wcr.reshape(64, 128, R, 8, 2).transpose(2, 4, 0, 3, 1)
            .reshape(R, 128, 8, 128).astype(BF))
        # wnp [R, 128=b, CH, 64=m]: per-channel squares summed on-chip
        wnp = np.ascontiguousarray(
            wcr.reshape(64, 128, R, CH).transpose(2, 1, 3, 0).astype(BF))
        # bcol [128=(c4,v), 32=j]: bias for channel CH*rr + 4q + c4, j=4rr+q
        bcol = np.zeros((128, 32), np.float32)
        bsl = b[sl]
        for j in range(32):
            rr, q = j // 4, j % 4
            for c4 in range(4):
                ch = CH * rr + 4 * q + c4
                bcol[c4 * 32:(c4 + 1) * 32, j] = bsl[ch]
        m = {"xprep": xprep, "wprep": wprep, "wnp": wnp, "bcol": bcol}
        m.update(consts)
        in_maps.append(m)
    return in_maps


def kernel(x, w, b):
    if "nc" not in _CACHE:
        _CACHE["consts"] = _consts()
        _CACHE["nc"] = _build()
    nc = _CACHE["nc"]

    x = np.ascontiguousarray(np.asarray(x, dtype=np.float32))
    w = np.ascontiguousarray(np.asarray(w, dtype=np.float32))
    b = np.ascontiguousarray(np.asarray(b, dtype=np.float32))
    in_maps = _prep(x, w, b)

    trace = os.environ.get("BASS_CONV_TRACE", "") == "1"
    last_err = None
    for _attempt in range(3):
        try:
            res = run_bass_kernel_spmd(nc, in_maps, core_ids=list(range(NCORES)),
                                       trace=trace)
            break
        except Exception as e:   # transient NRT device errors happen under axon
            last_err = e
    else:
        raise last_err
    if trace and res.exec_time_ns is not None:
        print(f"HW exec time: {res.exec_time_ns} ns")
        kernel.last_exec_ns = res.exec_time_ns
    # EI mask: +1 for excitatory, -1 for the last NUM_INH inhibitory units —
    # constant per 128-channel core slice, applied here at gather.
    ei = np.concatenate([np.ones(C - NUM_INH, np.float32),
                         -np.ones(NUM_INH, np.float32)])
    outs = []
    for i in range(NCORES):
        o = np.asarray(res.results[i]["outT"])          # [CPC, T]
        sgn = ei[CPC * i]                               # whole slice is +-1
        outs.append(np.ascontiguousarray((o * sgn).T))
    return np.concatenate(outs, axis=1)
